# revision 48
# baseline (speedup 1.0000x reference)
"""Trainium2 Bass kernel for the SE + patch-correlation-attention + down-conv module.

Sharding (8 cores): split the 96 image rows into 8 slabs of 12 rows. Each core:
  1. computes SE global-pool partials  -> tiny AllReduce (256 floats)
  2. computes SE gate y, S = sigmoid(x*y) on its 14-row halo slab
  3. patch correlation (9 shifts, bf16 DVE products + PE all-ones reduction
     that lands the channel-sum REPLICATED across all 128 partitions, so the
     softmax needs no partition-broadcast DMA), chunked exp on ScalarE, the
     softmax denominator accumulated on PE with the same all-ones stationary,
     then the weighted sum against raw bf16 x patches (the SE gate y and the
     1/denominator are folded into one per-channel scale applied at the end)
  4. AllToAll (bf16) redistributes the attention output so that core k holds
     the 32-column slice [32k,32k+32) of the .view()-scrambled Z2 matrix
  5. local 256x256 down matmul, InstanceNorm partials -> tiny AllReduce,
     normalize + LeakyReLU, write its (256, 32, 36) output slice
Host gathers the 8 slices and permutes into (1, 256, 96, 96).

Layout notes: S/X slabs are (128, 14, 100) bf16 with image column j at slab
column j+2, zero padding around. A second copy shifted by one element (j at
column j+3) keeps every shifted view 4-byte aligned so bf16 tensor_tensor
products run in the DVE 2x perf mode.
"""
import numpy as np

C, H, W, M = 256, 96, 96, 8
RPC = H // M          # 12 rows per core
P = RPC * W           # 1152 positions per core
SLAB = RPC + 2        # 14 rows incl. halo
WP = 100              # padded slab width (even stride, j0 at col 2)
HW = H * W            # 9216
# dj=0 shifts first: their products read only the primary S copy, so phase 2
# starts before the +1-shifted copies are written (the d ordering is shared
# consistently by both phases; softmax sums are order-independent)
SHIFTS = [(di, dj) for dj in (0, -1, 1) for di in (-1, 0, 1)]
CHUNKS = [(0, 512), (512, 512), (1024, 128)]   # psum-bank-aligned matmul N-chunks

_cache = {}


def _build():
    import concourse.bass as bass
    from concourse import bacc
    import concourse.mybir as mybir
    from concourse.tile import TileContext
    from concourse.masks import make_identity

    fp32 = mybir.dt.float32
    bf16 = mybir.dt.bfloat16
    AF = mybir.ActivationFunctionType
    Alu = mybir.AluOpType
    GROUPS = [list(range(M))]

    nc = bacc.Bacc()

    xs = nc.declare_dram_parameter("xs", [C, SLAB, W], bf16, isOutput=False)
    xf = nc.declare_dram_parameter("xf", [C, H, W], bf16, isOutput=False)
    msk = nc.declare_dram_parameter("msk", [128, 2], fp32, isOutput=False)
    w1t = nc.declare_dram_parameter("w1t", [C, 16], fp32, isOutput=False)
    b1 = nc.declare_dram_parameter("b1", [16, 1], fp32, isOutput=False)
    w2t = nc.declare_dram_parameter("w2t", [16, C], fp32, isOutput=False)
    b2 = nc.declare_dram_parameter("b2", [C, 1], fp32, isOutput=False)
    dwt = nc.declare_dram_parameter("dwt", [C, C], bf16, isOutput=False)
    outp = nc.declare_dram_parameter("out", [C, 32, 36], fp32, isOutput=True)

    warm_in = nc.dram_tensor("warm_in", [M, 1], fp32)
    warm_out = nc.dram_tensor("warm_out", [M, 1], fp32)
    a2a_in = nc.dram_tensor("a2a_in", [M, 32, P], bf16)
    a2a_out = nc.dram_tensor("a2a_out", [M, 32, P], bf16)
    st_part = nc.dram_tensor("st_part", [128, 4], fp32)
    st_sum = nc.dram_tensor("st_sum", [128, 4], fp32, addr_space="Shared")
    dma = nc.default_dma_engine

    with TileContext(nc) as tc:
        with (
            tc.tile_pool(name="const", bufs=1) as cp,
            tc.tile_pool(name="sb", bufs=1) as sp,
            tc.tile_pool(name="work", bufs=6) as wp,
        ):
            # ---------- warm-up collective ----------
            # The first ncfw op after the ~21-65us init barrier pays a large
            # one-time setup (~30us observed on a first-op AllToAll). With the
            # SE pool computed locally, nothing needs ncfw until ~100us, so a
            # tiny dummy AllToAll fired at t=0 absorbs barrier + first-op
            # setup entirely in the background.
            nc.gpsimd.collective_compute(
                "AllToAll", Alu.bypass, replica_groups=GROUPS,
                ins=[warm_in[:, :]], outs=[warm_out[:, :]],
            )

            # ---------- load x slab (bf16, host-cast) straight into the
            # padded dual-copy layout; ct0 on the sync HWDGE queue, ct1 on
            # the GpSimd SWDGE queue so the Scalar queue stays free for the
            # pool-reduce activations.
            xb_sb = [sp.tile([128, SLAB, WP], bf16, tag=f"xb{ct}", name=f"xb{ct}") for ct in range(2)]
            xb2_sb = [sp.tile([128, SLAB, WP], bf16, tag=f"xb2{ct}", name=f"xb2{ct}") for ct in range(2)]
            dma.dma_start(out=xb_sb[0][:, :, 2:98], in_=xs[0:128, :, :])
            dma.dma_start(out=xb2_sb[0][:, :, 3:99], in_=xs[0:128, :, :])
            nc.gpsimd.dma_start(out=xb_sb[1][:, :, 2:98], in_=xs[128:256, :, :])
            nc.gpsimd.dma_start(out=xb2_sb[1][:, :, 3:99], in_=xs[128:256, :, :])

            # ---------- SE pool: every core streams the FULL x and reduces it
            # locally on ScalarE (accum_out), replacing the SE AllReduce.
            # The ~26us of HBM reads run during the otherwise-idle ~21-60us
            # ncfw init/barrier window, so the gate is ready ~40us earlier
            # and the AllToAll (now the first ncfw op) lands after the
            # barrier has already retired.
            sp4 = sp.tile([128, 8], fp32, tag="sp4")
            pool_scr = sp.tile([128, 24, W], bf16, tag="poolscr")

            # ---------- constants (emitted before the AR wait fills DVE) ----
            ident = cp.tile([128, 128], bf16)
            make_identity(nc, ident)
            ones128 = cp.tile([128, 128], bf16)
            nc.vector.memset(ones128, 1.0)

            eps_sb = cp.tile([128, 1], fp32)
            nc.vector.memset(eps_sb, 1e-5)
            msk_sb = cp.tile([128, 2], fp32)
            dma.dma_start(out=msk_sb, in_=msk[:, :])
            b1_sb = cp.tile([16, 1], fp32)
            dma.dma_start(out=b1_sb, in_=b1[:, :])
            b2_sb = cp.tile([128, 2], fp32)
            w1_sb = [cp.tile([128, 16], fp32, tag=f"w1_{ct}", name=f"w1_{ct}") for ct in range(2)]
            dw_sb = [cp.tile([128, C], bf16, tag=f"dw_{ct}", name=f"dw_{ct}") for ct in range(2)]
            for ct in range(2):
                dma.dma_start(out=b2_sb[:, ct : ct + 1], in_=b2[128 * ct : 128 * ct + 128, :])
                dma.dma_start(out=w1_sb[ct], in_=w1t[128 * ct : 128 * ct + 128, :])
            w2_sb = cp.tile([16, C], fp32)
            dma.dma_start(out=w2_sb, in_=w2t[:, :])

            # ---------- full-x pool chunks (2 HWDGE queues, ScalarE reduce) --
            # ct0 chunks reduce on ScalarE (accum_out), ct1 on VectorE, so the
            # two reduce streams run concurrently with the two DMA queues
            # all chunk DMAs issue before any reduce, so no DMA is FIFO-
            # trapped behind a 2.2us reduce on its issuing queue. ct0 chunks
            # ride the otherwise-empty Scalar HWDGE queue; their reduces run
            # on ScalarE afterwards, ct1's on VectorE.
            with tc.tile_pool(name="xfp", bufs=8) as xfp:
                xfcs = []
                for c in range(4):
                    for ct in range(2):
                        xfc = xfp.tile([128, 24, W], bf16, tag="xfc", name=f"xf{ct}{c}")
                        eng = nc.scalar if ct == 0 else nc.gpsimd
                        eng.dma_start(
                            out=xfc,
                            in_=xf[128 * ct : 128 * ct + 128, 24 * c : 24 * c + 24, :],
                        )
                        xfcs.append((ct, c, xfc))
                for ct, c, xfc in xfcs:
                    if ct == 0:
                        nc.scalar.activation(
                            out=pool_scr, in_=xfc, func=AF.Identity,
                            accum_out=sp4[:, c : c + 1],
                        )
                    else:
                        nc.vector.tensor_reduce(
                            out=sp4[:, 4 + c : 5 + c], in_=xfc,
                            axis=mybir.AxisListType.XY, op=Alu.add,
                        )
            # preload the Sigmoid ACT table during the pool tail so the table
            # swap is off the gate's critical path
            nc.scalar.activation(out=pool_scr[:, 0, 0:1], in_=eps_sb, func=AF.Sigmoid)
            # down weights load after the pool stream (not needed until ~120us)
            for ct in range(2):
                dma.dma_start(out=dw_sb[ct], in_=dwt[128 * ct : 128 * ct + 128, :])

            # ---------- S tiles + pad-column zeroing ----------
            s_sb = [sp.tile([128, SLAB, WP], bf16, tag=f"s{ct}", name=f"s{ct}") for ct in range(2)]
            s2_sb = [sp.tile([128, SLAB, WP], bf16, tag=f"s2{ct}", name=f"s2{ct}") for ct in range(2)]
            for ct in range(2):
                # only the pad columns the dj=+-1 views read need zeroing
                nc.vector.memset(s2_sb[ct][:, :, 2:3], 0.0)
                nc.vector.memset(s2_sb[ct][:, :, 99:100], 0.0)
                nc.vector.memset(xb2_sb[ct][:, :, 2:3], 0.0)
                nc.vector.memset(xb2_sb[ct][:, :, 99:100], 0.0)

            ssum_sb = sp.tile([128, 2], fp32, tag="sesum")
            for ct in range(2):
                nc.vector.tensor_reduce(
                    out=ssum_sb[:, ct : ct + 1], in_=sp4[:, 4 * ct : 4 * ct + 4],
                    axis=mybir.AxisListType.X, op=Alu.add,
                )

            # ---------- SE gate ----------
            with tc.tile_pool(name="ps_se", bufs=1, space="PSUM") as pse:
                h_ps = pse.tile([16, 1], fp32)
                for ct in range(2):
                    nc.tensor.matmul(
                        h_ps, w1_sb[ct], ssum_sb[:, ct : ct + 1],
                        start=(ct == 0), stop=(ct == 1),
                    )
                h_sb = sp.tile([16, 1], fp32)
                nc.scalar.activation(out=h_sb, in_=h_ps, func=AF.Relu,
                                     bias=b1_sb, scale=1.0 / HW)
                y_ps = pse.tile([128, 2], fp32)
                y_sb = sp.tile([128, 2], fp32, tag="ygate")
                for ct in range(2):
                    nc.tensor.matmul(
                        y_ps[:, ct : ct + 1], w2_sb[:, 128 * ct : 128 * ct + 128], h_sb,
                        start=True, stop=True,
                    )
                    nc.scalar.activation(out=y_sb[:, ct : ct + 1], in_=y_ps[:, ct : ct + 1],
                                         func=AF.Sigmoid, bias=b2_sb[:, ct : ct + 1], scale=1.0)

            # ---------- S map (sigmoid of gated x) ----------
            for ct in range(2):
                nc.scalar.activation(
                    out=s_sb[ct][:, :, 2:98], in_=xb_sb[ct][:, :, 2:98],
                    func=AF.Sigmoid, scale=y_sb[:, ct : ct + 1],
                )
                # zero invalid halo rows (top/bottom image edge)
                nc.vector.tensor_scalar(
                    out=s_sb[ct][:, 0, 2:98], in0=s_sb[ct][:, 0, 2:98],
                    scalar1=msk_sb[:, 0:1], scalar2=None, op0=Alu.mult,
                )
                nc.vector.tensor_scalar(
                    out=s_sb[ct][:, 13, 2:98], in0=s_sb[ct][:, 13, 2:98],
                    scalar1=msk_sb[:, 1:2], scalar2=None, op0=Alu.mult,
                )
                nc.vector.tensor_copy(out=s2_sb[ct][:, :, 3:99], in_=s_sb[ct][:, :, 2:98])

            def sview(ct, di, dj):
                """4B-aligned view of S shifted by (di, dj), rows 1..12."""
                if dj == 0:
                    return s_sb[ct][:, 1 + di : 13 + di, 2:98]
                return s2_sb[ct][:, 1 + di : 13 + di, 3 + dj : 99 + dj]

            def xbview(ct, di, dj):
                if dj == 0:
                    return xb_sb[ct][:, 1 + di : 13 + di, 2:98]
                return xb2_sb[ct][:, 1 + di : 13 + di, 3 + dj : 99 + dj]

            # ---------- phase 2: correlation, exp, denominator ----------
            # A_rep[d] = ones128.T @ (S*S_d): every psum partition holds the
            # channel sum, so exp lands pre-broadcast with zero DMA. The
            # denominator accumulates on PE with the same all-ones stationary
            # (values are 128x the true sums; folded out via the 128* in the
            # final gate scale).
            exp_sb = [sp.tile([128, P], bf16, tag=f"exp{d}", name=f"exp{d}") for d in range(9)]
            rec_sb = sp.tile([128, P], fp32, tag="recs")
            recy_sb = [sp.tile([128, P], fp32, tag=f"recy{ct}", name=f"recy{ct}") for ct in range(2)]
            with (
                tc.tile_pool(name="ps_a", bufs=4, space="PSUM") as pa,
                tc.tile_pool(name="ps_den", bufs=1, space="PSUM") as pd,
                tc.tile_pool(name="ps_warm", bufs=1, space="PSUM") as pw,
            ):
                den_ps = pd.tile([128, P], fp32, tag="den", name="den")
                for d, (di, dj) in enumerate(SHIFTS):
                    prods = []
                    for ct in range(2):
                        prod = wp.tile([128, P], bf16, tag="prod")
                        pv = prod.rearrange("c (r w) -> c r w", w=W)
                        nc.vector.tensor_tensor(
                            out=pv,
                            in0=s_sb[ct][:, 1:13, 2:98],
                            in1=sview(ct, di, dj),
                            op=Alu.mult,
                        )
                        prods.append(prod)
                    # pre-add the channel halves on DVE: halves the phase-2
                    # matmul + LDWEIGHTS count on the PE pacer
                    psum_t = wp.tile([128, P], bf16, tag="prod", name=f"psum{d}")
                    nc.vector.tensor_tensor(out=psum_t, in0=prods[0],
                                            in1=prods[1], op=Alu.add)
                    # single-bank A chunks (bufs=4): d+1's matmuls start as
                    # soon as a bank frees instead of waiting for ALL of d's
                    # exps, breaking the per-d serial chain
                    for (o, n) in CHUNKS:
                        Ac = pa.tile([128, 512], fp32, tag="achunk",
                                     name=f"A{d}_{o}")
                        nc.tensor.matmul(
                            Ac[:, 0:n], ones128, psum_t[:, o : o + n],
                            start=True, stop=True,
                        )
                        nc.scalar.activation(out=exp_sb[d][:, o : o + n],
                                             in_=Ac[:, 0:n],
                                             func=AF.Exp, scale=1.0 / C)
                        # denominator accumulates on PE with the same all-ones
                        # stationary (values are 128x; folded out below)
                        nc.tensor.matmul(
                            den_ps[:, o : o + n], ones128, exp_sb[d][:, o : o + n],
                            start=(d == 0), stop=(d == 8),
                        )
                # ~1us of throwaway matmuls bridging the softmax gap so the
                # PE HAM clock-gate stays at full rate for phase 3 (an idle
                # gap >3.4us would re-throttle it to half clock)
                warm_ps = pw.tile([128, 512], fp32, tag="warm", name="warm")
                for _ in range(4):
                    nc.tensor.matmul(warm_ps, ones128, exp_sb[8][:, 0:512],
                                     start=True, stop=True)
                warm_rd = sp.tile([128, 1], fp32, tag="warmrd")
                nc.vector.tensor_copy(out=warm_rd, in_=warm_ps[:, 0:1])
                # den_ps = 128 * sum_d exp_d ; fold the 128 into the gate scale
                nc.vector.reciprocal_approx_fast(out=rec_sb, in_=den_ps)
            for ct in range(2):
                nc.vector.tensor_scalar(
                    out=recy_sb[ct], in0=rec_sb,
                    scalar1=y_sb[:, ct : ct + 1], scalar2=128.0,
                    op0=Alu.mult, op1=Alu.mult,
                )

            # ---------- phase 3: weighted sum  out[c,p] = y*sum_d a_d*X_d ----
            with tc.tile_pool(name="ps_acc", bufs=1, space="PSUM") as pacc:
                for ct in range(2):
                    acc = pacc.tile([128, P], fp32, tag="dzacc", name=f"acc{ct}")
                    for d, (di, dj) in enumerate(SHIFTS):
                        prod = wp.tile([128, P], bf16, tag="prod")
                        pv = prod.rearrange("c (r w) -> c r w", w=W)
                        nc.vector.tensor_tensor(
                            out=pv,
                            in0=xbview(ct, di, dj),
                            in1=exp_sb[d].rearrange("c (r w) -> c r w", w=W),
                            op=Alu.mult,
                        )
                        for (o, n) in CHUNKS:
                            nc.tensor.matmul(
                                acc[:, o : o + n], ident, prod[:, o : o + n],
                                start=(d == 0), stop=(d == 8),
                            )
                    oat = sp.tile([128, P], bf16, tag=f"oat{ct}", name=f"oat{ct}")
                    nc.vector.tensor_tensor(out=oat, in0=acc, in1=recy_sb[ct], op=Alu.mult)
                    eng = dma if ct == 0 else nc.scalar
                    eng.dma_start(out=a2a_in[4 * ct : 4 * ct + 4, :, :], in_=oat)

            # ---------- AllToAll ----------
            nc.gpsimd.collective_compute(
                "AllToAll", Alu.bypass, replica_groups=GROUPS,
                ins=[a2a_in[:, :, :]], outs=[a2a_out[:, :, :]],
            )

            # ---------- down matmul on the scrambled layout ----------
            # rhs[ch, s, t] = a2a_out[ch//32, s, 36*(ch%32)+t]
            rhs_sb = [sp.tile([128, 32, 36], bf16, tag=f"rhs{kt}", name=f"rhs{kt}") for kt in range(2)]
            v = a2a_out.rearrange("j s (b t) -> j b s t", t=36)
            for kt in range(2):
                for a in range(4):
                    eng = dma if a % 2 == 0 else nc.scalar
                    eng.dma_start(
                        out=rhs_sb[kt][32 * a : 32 * a + 32, :, :],
                        in_=v[4 * kt + a],
                    )

            stat_sb = sp.tile([128, 4], fp32, tag="stat")
            sq_scr = wp.tile([128, P], fp32, tag="sqscr")
            zo_sb = [sp.tile([128, P], fp32, tag=f"zo{mt}", name=f"zo{mt}") for mt in range(2)]
            with tc.tile_pool(name="ps_z", bufs=1, space="PSUM") as pz:
                z_ps = [pz.tile([128, P], fp32, tag=f"z{mt}", name=f"z{mt}") for mt in range(2)]
                for mt in range(2):
                    for (o, n) in CHUNKS:
                        for kt in range(2):
                            nc.tensor.matmul(
                                z_ps[mt][:, o : o + n],
                                dw_sb[kt][:, 128 * mt : 128 * mt + 128],
                                rhs_sb[kt].rearrange("c s t -> c (s t)")[:, o : o + n],
                                start=(kt == 0), stop=(kt == 1),
                            )
                    # IN stats partials
                    nc.vector.tensor_reduce(
                        out=stat_sb[:, mt : mt + 1], in_=z_ps[mt],
                        axis=mybir.AxisListType.X, op=Alu.add,
                    )
                    nc.scalar.activation(
                        out=sq_scr, in_=z_ps[mt], func=AF.Square,
                        accum_out=stat_sb[:, 2 + mt : 3 + mt],
                    )
                dma.dma_start(out=st_part[:, :], in_=stat_sb)
                nc.gpsimd.collective_compute(
                    "AllReduce", Alu.add, replica_groups=GROUPS,
                    ins=[st_part[:, :]], outs=[st_sum[:, :]],
                )
                gl_sb = sp.tile([128, 4], fp32, tag="glstat")
                dma.dma_start(out=gl_sb, in_=st_sum[:, :])

                # mu = sum/HW ; var = sumsq/HW - mu^2 ; inv = 1/sqrt(var+eps)
                # batched over both channel tiles: cols [mt] = mu, [2+mt] = var
                ins_sb = sp.tile([128, 8], fp32, tag="instat")
                mu2 = ins_sb[:, 0:2]
                e22 = ins_sb[:, 2:4]
                inv2 = ins_sb[:, 4:6]
                nmi2 = ins_sb[:, 6:8]
                nc.vector.tensor_scalar(out=mu2, in0=gl_sb[:, 0:2],
                                        scalar1=1.0 / HW, scalar2=None, op0=Alu.mult)
                nc.vector.tensor_scalar(out=e22, in0=gl_sb[:, 2:4],
                                        scalar1=1.0 / HW, scalar2=None, op0=Alu.mult)
                nc.vector.tensor_tensor(out=inv2, in0=mu2, in1=mu2, op=Alu.mult)
                nc.vector.tensor_tensor(out=e22, in0=e22, in1=inv2, op=Alu.subtract)
                nc.scalar.activation(out=e22, in_=e22, func=AF.Sqrt, bias=eps_sb, scale=1.0)
                nc.vector.reciprocal(out=inv2, in_=e22)
                # nmi = -mu * inv  (bias for the fused Prelu normalize)
                nc.vector.scalar_tensor_tensor(out=nmi2, in0=mu2, scalar=-1.0,
                                               in1=inv2, op0=Alu.mult, op1=Alu.mult)
                for mt in range(2):
                    # LeakyReLU((z - mu) * inv) fused on ScalarE:
                    #   prelu(z*inv + (-mu*inv), alpha=0.2)
                    nc.scalar.activation(
                        out=zo_sb[mt], in_=z_ps[mt], func=AF.Prelu,
                        bias=ins_sb[:, 6 + mt : 7 + mt],
                        scale=ins_sb[:, 4 + mt : 5 + mt], alpha=0.2,
                    )
                    dma.dma_start(
                        out=outp[128 * mt : 128 * mt + 128, :, :],
                        in_=zo_sb[mt].rearrange("c (s t) -> c s t", t=36),
                    )
    nc.compile()
    return nc


def _get_nc():
    if "nc" not in _cache:
        _cache["nc"] = _build()
    return _cache["nc"]


def _shard_inputs(x, se_w1, se_b1, se_w2, se_b2, down_w):
    import ml_dtypes

    x = np.ascontiguousarray(np.asarray(x, np.float32))[0]          # (C, H, W)
    w1t = np.ascontiguousarray(np.asarray(se_w1, np.float32).T)     # (C, 16)
    b1 = np.ascontiguousarray(np.asarray(se_b1, np.float32)[:, None])
    w2t = np.ascontiguousarray(np.asarray(se_w2, np.float32).T)     # (16, C)
    b2 = np.ascontiguousarray(np.asarray(se_b2, np.float32)[:, None])
    dwt = np.ascontiguousarray(
        np.asarray(down_w, np.float32).T.astype(ml_dtypes.bfloat16)
    )                                                               # (C, C) bf16

    xf16 = np.ascontiguousarray(x.astype(ml_dtypes.bfloat16))
    in_maps = []
    for k in range(M):
        slab = np.zeros((C, SLAB, W), ml_dtypes.bfloat16)
        lo, hi = RPC * k - 1, RPC * k + RPC + 1
        clo, chi = max(lo, 0), min(hi, H)
        slab[:, clo - lo : clo - lo + (chi - clo), :] = x[:, clo:chi, :].astype(
            ml_dtypes.bfloat16
        )
        msk = np.ones((128, 2), np.float32)
        if k == 0:
            msk[:, 0] = 0.0
        if k == M - 1:
            msk[:, 1] = 0.0
        in_maps.append({
            "xs": slab, "xf": xf16, "msk": msk, "w1t": w1t, "b1": b1,
            "w2t": w2t, "b2": b2, "dwt": dwt,
        })
    return in_maps


def _gather(results):
    R = np.stack([np.asarray(r["out"], np.float32) for r in results])  # (8, 256, 32, 36)
    return np.ascontiguousarray(
        R.transpose(1, 3, 0, 2).reshape(1, C, H, W).astype(np.float32)
    )


def kernel(x, se_w1, se_b1, se_w2, se_b2, down_w, _trace=False):
    from concourse.bass_utils import run_bass_kernel_spmd

    nc = _get_nc()
    in_maps = _shard_inputs(x, se_w1, se_b1, se_w2, se_b2, down_w)
    res = run_bass_kernel_spmd(nc, in_maps, core_ids=list(range(M)), trace=_trace)
    out = _gather(res.results)
    if _trace:
        kernel.last_results = res
    return out
